# revision 17
# baseline (speedup 1.0000x reference)
"""AGCA (adaptive graph channel attention) distributed Bass kernel for TRN2.

Reference computation (per batch row b):
    y   = mean(x[b], axis=(H,W))                    # [CIN]
    y1  = W1 @ y                                    # [HIDE]
    A1  = softmax(w2 * y1)                          # [HIDE]
    y2  = y1 * A1 + y1 @ A2
    y3  = relu(w3 * y2)
    out = sigmoid(W4 @ y3)                          # [OP]

Sharding: pure data-parallel over batch. Each of the 8 cores handles
B/8 = 8 batch rows end-to-end; the tiny params are replicated. No
collectives. The kernel is memory-bound on streaming x (64 MiB/core).

Per-core dataflow (v2 — dual-ring HWDGE stream):
  - x shard viewed as [BL=8, CT=4, 128, 4096]; streamed as 36 units
    (30 full [128,1,4096] f32 2-MiB units ct-major, one full unit for
    row (ct3,b6) reduced on ACT, and the last row (ct3,b7) tapered
    into 5 descending hw-chunks so the final reduce lands ~0.4us after
    the last byte).
  - Units alternate between the two HWDGE rings (sync=even, scalar=odd)
    in plain f32 (no cast; HWDGE has ~0.6us first-byte vs SWDGE's
    ~2.6us Q7 emission, and each ring's InstDMACopy is split across all
    16 SDMA engines). Measured per-engine ceiling ~26.6 GB/s; the
    stream runs at ~425 GB/s/core, at the 16-engine cap and essentially
    at the HBM roofline. 6 rotating SBUF buffers (96 KiB/partition);
    issue of unit i gates on the reduce of unit i-6.
  - Known hardware lottery (measured, both DGE paths equally): some
    runs have 1-3 cores with one SDMA engine at ~0.81x (bursty
    descriptor stalls), and some runs have whole HBM-stack pairs at
    ~0.81-0.88x. Per-engine byte targeting is impossible (the DGE
    splits every transfer evenly), and the slow set moves between
    runs, so the schedule stays symmetric; the graded max-core time is
    dominated by that lottery.
  - The vector engine sum-reduces each unit along the free axis into
    yt [128c, CT, 8b] (f32; the 1/4096 mean scale is folded into W1 on
    the host). DVE sits at ~60% busy at line rate.
  - Params ride the otherwise-idle gpsimd SWDGE queue; the exp/tanh
    LUT set is preloaded on ACT during the stream.
  - Per channel tile, its W1 matmuls run mid-stream on the tensor
    engine (y1 [8,128] and y1T [128,8] layouts both computed so softmax
    runs along the free axis). The epilogue reads y1 straight from
    PSUM: exp with fused accum (softmax denominator) on ACT, the
    normalize/A2/relu chain on DVE+PE, and sigmoid as
    0.5*tanh(z/2)+0.5 (tanh shares the exp LUT set). Output halves go
    out on the two rings so the second half's tanh overlaps the first
    half's HBM write receipt.
"""

import numpy as np

import concourse.bass as bass
import concourse.mybir as mybir
from concourse.bass_utils import run_bass_kernel_spmd


def _install_ntff_shim():
    """Fill in the optional antenv.axon_hooks module if the image lacks it,
    so run_bass_kernel_spmd(trace=True) (or BASS_TRACE=1) can drive NTFF
    profiling through libaxon_pjrt.so instead of crashing on the import.
    No-op when the module exists or the axon .so is unavailable."""
    import sys as _sys
    import types as _types

    if "antenv.axon_hooks" in _sys.modules:
        return
    try:
        import antenv  # noqa: F401
        import importlib.util as _ilu

        if _ilu.find_spec("antenv.axon_hooks") is not None:
            return
        mod = _types.ModuleType("antenv.axon_hooks")
        _hook = [None]
        mod.set_axon_ntff_profile_hook = lambda h: _hook.__setitem__(0, h)
        mod.get_axon_ntff_profile_hook = lambda: _hook[0]
        try:
            from trn_agent_boot.trn_boot import _ntff_profile_via_ctypes

            mod.set_axon_ntff_profile_hook(
                _ntff_profile_via_ctypes("/opt/axon/libaxon_pjrt.so")
            )
        except Exception:
            pass  # hook stays None; bass_utils logs and skips tracing
        _sys.modules["antenv.axon_hooks"] = mod
        antenv.axon_hooks = mod
    except Exception:
        pass


_install_ntff_shim()

F32 = mybir.dt.float32

B, CIN, H, W = 64, 512, 64, 64
HW = H * W          # 4096
NCORES = 8
BL = B // NCORES    # 8 batch rows per core
CT = CIN // 128     # 4 channel tiles
HIDE = 128
OP = 512
NBUF = 6            # rotating f32 stream buffers (16 KiB/partition each)

# hw tapers of the last two batch rows. DVE's f32 reduce rate
# (~4.4us per 2-MiB unit) is ~90% of the per-unit arrival rate, so the
# full-unit reduces alternate DVE/ACT (~45% each) and the final two
# rows are chunked with reduces interleaved across both engines so the
# last combine lands <1us after the last byte.
TAPER6 = [(1024, "V"), (1024, "V"), (1024, "V"), (1024, "V")]
TAPER7 = [(1536, "A"), (1024, "V"), (768, "A"), (512, "V"), (256, "A")]
assert sum(n for n, _ in TAPER6) == HW and sum(n for n, _ in TAPER7) == HW
NT6 = len(TAPER6)
NT7 = len(TAPER7)


def make_units():
    """39 stream units: (eng, ct, b, hw0, nhw, dst).

    eng 'V' (DVE reduce_sum) or 'A' (ACT Copy+accum reduce); dst is
    ('yt',) for a full row, ('y6', k) / ('y7', k) for taper partials.
    Full units alternate V/A by index so both reduce engines sit near
    45% of the stream rate.
    """
    units = []

    def full(ct, b):
        eng = "V" if len(units) % 2 == 0 else "A"
        units.append((eng, ct, b, 0, HW, ("yt",)))

    for ct in range(CT - 1):
        for b in range(BL):
            full(ct, b)
    for b in range(BL - 2):
        full(CT - 1, b)
    hw0 = 0
    for k, (nhw, eng) in enumerate(TAPER6):
        units.append((eng, CT - 1, BL - 2, hw0, nhw, ("y6", k)))
        hw0 += nhw
    hw0 = 0
    for k, (nhw, eng) in enumerate(TAPER7):
        units.append((eng, CT - 1, BL - 1, hw0, nhw, ("y7", k)))
        hw0 += nhw
    return units


def build_nc():
    nc = bass.Bass(enable_partition_id=False, monotonic_sem_count=0)
    BF16 = mybir.dt.bfloat16

    x_e = nc.declare_dram_parameter("x", [BL, CT, 128, HW], F32, isOutput=False)
    w1t_e = nc.declare_dram_parameter("w1t", [128, CT, HIDE], F32, isOutput=False)
    a2_e = nc.declare_dram_parameter("a2", [HIDE, HIDE], BF16, isOutput=False)
    w4t_e = nc.declare_dram_parameter("w4t", [HIDE, OP], BF16, isOutput=False)
    scal_e = nc.declare_dram_parameter("scal", [BL, 2], F32, isOutput=False)
    eye_e = nc.declare_dram_parameter("eye8", [BL, BL], BF16, isOutput=False)
    out_e = nc.declare_dram_parameter("out", [BL, OP], F32, isOutput=True)

    Exp = mybir.ActivationFunctionType.Exp
    Tanh = mybir.ActivationFunctionType.Tanh
    Copy = mybir.ActivationFunctionType.Copy
    Relu = mybir.ActivationFunctionType.Relu

    units = make_units()
    NU = len(units)          # 39
    # consumer bookkeeping: after consuming unit i, which sem has what
    vcnt = acnt = 0          # DVE (red_sem) / ACT (act_sem) increments
    cons = []                # (eng, cumulative inc) consumer marker per unit
    ct_red = [0] * CT        # cumulative red count through each ct's units
    ct_act = [0] * CT
    i6 = i7 = 0              # DVE chunk position where combine6 fits
    for i, (eng, ct, b, hw0, nhw, dst) in enumerate(units):
        if eng == "V":
            vcnt += 1
            cons.append(("V", vcnt))
        else:
            acnt += 1
            cons.append(("A", acnt))
        ct_red[ct] = vcnt
        ct_act[ct] = acnt
    NVS = vcnt               # stream reduces on DVE (21)
    NAS = acnt               # stream reduces on ACT (18)
    # DVE order: stream units in order, then combine6 is inserted right
    # after the last row-6 chunk (before the row-7 DVE chunks), and
    # combine7 comes last.
    RCOMB6 = sum(1 for e, ct, b, h, n, d in units
                 if e == "V" and not (d[0] == "y7")) + 1
    RCOMB7 = NVS + 2         # after all DVE stream reduces + combine6
    # act count after the last row-6-feeding ACT unit (none: row6 all V)
    A_ROW6 = max((cons[i][1] for i, u in enumerate(units)
                  if u[5][0] == "y6" and u[0] == "A"), default=0)
    A_ROW7 = max(cons[i][1] for i, u in enumerate(units)
                 if u[5][0] == "y7" and u[0] == "A")
    # DVE epilogue increments
    R_Y1TS = RCOMB7 + 1      # y1ts bf16 copy
    R_RECIP = RCOMB7 + 2
    R_STT = RCOMB7 + 3
    R_Y2 = RCOMB7 + 4
    R_OUTH1 = RCOMB7 + 5
    # ACT epilogue increments
    A_EXP = NAS + 1
    A_RELU = NAS + 2
    A_TANH1 = NAS + 3
    A_TANH2 = NAS + 4
    A_OUTH2 = NAS + 5
    NPARAM = 5 * 16

    from contextlib import ExitStack

    with ExitStack() as ctx:
        bufs = [
            ctx.enter_context(nc.sbuf_tensor(f"buf{j}", [128, 1, HW], F32))
            for j in range(NBUF)
        ]
        yt = ctx.enter_context(nc.sbuf_tensor("yt", [128, CT, BL], F32))
        ytx6 = ctx.enter_context(nc.sbuf_tensor("ytx6", [128, NT6], F32))
        ytx7 = ctx.enter_context(nc.sbuf_tensor("ytx7", [128, NT7], F32))
        waste = ctx.enter_context(nc.sbuf_tensor("waste", [128, 2, HW], BF16))
        w1ts = ctx.enter_context(nc.sbuf_tensor("w1ts", [128, CT, HIDE], F32))
        a2s = ctx.enter_context(nc.sbuf_tensor("a2s", [HIDE, HIDE], BF16))
        w4ts = ctx.enter_context(nc.sbuf_tensor("w4ts", [HIDE, OP], BF16))
        scals = ctx.enter_context(nc.sbuf_tensor("scals", [BL, 2], F32))
        eyes = ctx.enter_context(nc.sbuf_tensor("eyes", [BL, BL], BF16))
        de1 = ctx.enter_context(nc.sbuf_tensor("de1", [1, 1], F32))

        y1ts = ctx.enter_context(nc.sbuf_tensor("y1ts", [HIDE, BL], BF16))
        es = ctx.enter_context(nc.sbuf_tensor("es", [BL, HIDE], F32))
        ss = ctx.enter_context(nc.sbuf_tensor("ss", [BL, 1], F32))
        rs = ctx.enter_context(nc.sbuf_tensor("rs", [BL, 1], F32))
        t1s = ctx.enter_context(nc.sbuf_tensor("t1s", [BL, HIDE], F32))
        y2s = ctx.enter_context(nc.sbuf_tensor("y2s", [BL, HIDE], BF16))
        y3ts = ctx.enter_context(nc.sbuf_tensor("y3ts", [HIDE, BL], BF16))
        esig = ctx.enter_context(nc.sbuf_tensor("esig", [BL, OP], F32))
        outs = ctx.enter_context(nc.sbuf_tensor("outs", [BL, OP], F32))

        y1_ps = ctx.enter_context(nc.psum_tensor("y1_ps", [BL, HIDE], F32))
        y1t_ps = ctx.enter_context(nc.psum_tensor("y1t_ps", [HIDE, BL], F32))
        p2_ps = ctx.enter_context(nc.psum_tensor("p2_ps", [BL, HIDE], F32))
        y3t_ps = ctx.enter_context(nc.psum_tensor("y3t_ps", [HIDE, BL], F32))
        o_ps = ctx.enter_context(nc.psum_tensor("o_ps", [BL, OP], F32))

        dsems = [
            ctx.enter_context(nc.semaphore(f"dsem{j}")) for j in range(NBUF)
        ]
        out_sem = ctx.enter_context(nc.semaphore("out_sem"))
        param_sem = ctx.enter_context(nc.semaphore("param_sem"))
        red_sem = ctx.enter_context(nc.semaphore("red_sem"))
        pe_sem = ctx.enter_context(nc.semaphore("pe_sem"))
        act_sem = ctx.enter_context(nc.semaphore("act_sem"))

        slot_use = [0] * NBUF
        dwait = []               # per-unit (slot, wait_value)
        for i in range(NU):
            s = i % NBUF
            slot_use[s] += 1
            dwait.append((s, 16 * slot_use[s]))

        def issue_unit(eng, i):
            kind, ct, b, hw0, nhw, dst = units[i]
            if i >= NBUF:
                pk, pcnt = cons[i - NBUF]
                eng.wait_ge(red_sem if pk == "V" else act_sem, pcnt)
            s, _ = dwait[i]
            src = x_e[b:b + 1, ct, :, hw0:hw0 + nhw].rearrange(
                "b p w -> p b w")
            eng.dma_start(
                out=bufs[s][:, :, 0:nhw], in_=src
            ).then_inc(dsems[s], 16)

        def dst_ap(u):
            eng, ct, b, hw0, nhw, dst = u
            if dst[0] == "yt":
                return yt[:, ct, b:b + 1]
            if dst[0] == "y6":
                return ytx6[:, dst[1]:dst[1] + 1]
            return ytx7[:, dst[1]:dst[1] + 1]

        with nc.Block() as block:

            @block.gpsimd
            def _(gpsimd):
                # Tiny replicated params ride the otherwise-idle SWDGE
                # queue; done within ~15us, needed first at ~45us.
                gpsimd.dma_start(out=w1ts[:, :, :], in_=w1t_e[:, :, :]).then_inc(
                    param_sem, 16)
                gpsimd.dma_start(out=a2s[:, :], in_=a2_e[:, :]).then_inc(
                    param_sem, 16)
                gpsimd.dma_start(out=w4ts[:, :], in_=w4t_e[:, :]).then_inc(
                    param_sem, 16)
                gpsimd.dma_start(out=scals[:, :], in_=scal_e[:, :]).then_inc(
                    param_sem, 16)
                gpsimd.dma_start(out=eyes[:, :], in_=eye_e[:, :]).then_inc(
                    param_sem, 16)

            @block.sync
            def _(sync):
                for i in range(0, NU, 2):
                    issue_unit(sync, i)
                # Output first half once DVE computed it.
                sync.wait_ge(red_sem, R_OUTH1)
                sync.dma_start(
                    out=out_e[:, 0:OP // 2], in_=outs[:, 0:OP // 2]
                ).then_inc(out_sem, 16)
                sync.wait_ge(out_sem, 32)

            @block.scalar
            def _(scalar):
                # Preload the exp/tanh table set before the stream ends.
                c0 = nc.const_aps.tensor(0.0, (1, 1))
                scalar.activation(de1[:, :], c0, Exp)
                # Interleave this ring's DMA issues with ACT's share of
                # the reduces: reduce of unit u is placed after the issue
                # of unit u+5 (issue lookahead ~3 transfers per ring).
                # Two rotating waste regions with self-waits order the
                # region reuse through the ACT pipeline.
                events = sorted(
                    [("I", i, i) for i in range(1, NU, 2)]
                    + [("R", u + 5.5, u) for u in range(NU)
                       if units[u][0] == "A"],
                    key=lambda e: e[1],
                )
                region_last = [0, 0]
                acalls = 0
                for kind_e, _, u in events:
                    if kind_e == "I":
                        issue_unit(scalar, u)
                        continue
                    eng_u, ct, b, hw0, nhw, dst = units[u]
                    s, w = dwait[u]
                    scalar.wait_ge(dsems[s], w)
                    reg = acalls % 2
                    if region_last[reg] > 0:
                        scalar.wait_ge(act_sem, region_last[reg])
                    scalar.activation(
                        waste[:, reg, 0:nhw], bufs[s][:, 0, 0:nhw], Copy,
                        accum_out=dst_ap(units[u]),
                    ).then_inc(act_sem, 1)
                    acalls += 1
                    region_last[reg] = acalls
                # Epilogue: exp(w2*y1) with fused softmax denominator,
                # reading y1 straight out of PSUM.
                scalar.wait_ge(param_sem, NPARAM)
                scalar.wait_ge(pe_sem, 7)
                scalar.activation(
                    es[:, :], y1_ps[:, :], Exp, scale=scals[:, 0:1],
                    accum_out=ss[:, :],
                ).then_inc(act_sem, 1)
                scalar.wait_ge(pe_sem, 10)
                scalar.activation(y3ts[:, :], y3t_ps[:, :], Relu).then_inc(
                    act_sem, 1)
                # sigmoid(z) = 0.5*tanh(z/2) + 0.5 (tanh shares the exp
                # set). Column-half pipeline across ACT and DVE.
                scalar.wait_ge(pe_sem, 11)
                scalar.activation(
                    esig[:, 0:OP // 2], o_ps[:, 0:OP // 2], Tanh, scale=0.5
                ).then_inc(act_sem, 1)
                scalar.wait_ge(pe_sem, 12)
                scalar.activation(
                    esig[:, OP // 2:OP], o_ps[:, OP // 2:OP], Tanh, scale=0.5
                ).then_inc(act_sem, 1)
                scalar.wait_ge(act_sem, A_TANH2)  # esig h2 writeback
                scalar.activation(
                    outs[:, OP // 2:OP], esig[:, OP // 2:OP], Copy,
                    scale=0.5, bias=0.5,
                ).then_inc(act_sem, 1)
                # Second output half on this ring: overlaps the first
                # half's HBM write receipt. Self-wait on act_sem: the DMA
                # enqueue is async to the ACT compute pipeline, so program
                # order alone would race the copy above.
                scalar.wait_ge(act_sem, A_OUTH2)
                scalar.dma_start(
                    out=out_e[:, OP // 2:OP], in_=outs[:, OP // 2:OP]
                ).then_inc(out_sem, 16)

            @block.vector
            def _(vector):
                def combine6():
                    # yt[:, 3, 6] = sum(ytx6). Self-wait: engine
                    # pipelines do NOT interlock SBUF RAW between their
                    # own instructions — without it this reads ytx6
                    # before the chunk reduces' writebacks land.
                    vector.wait_ge(red_sem, RCOMB6 - 1)
                    vector.reduce_sum(
                        yt[:, CT - 1, BL - 2:BL - 1], ytx6[:, :],
                        axis=mybir.AxisListType.X,
                    ).then_inc(red_sem, 1)

                done6 = False
                for i in range(NU):
                    kind, ct, b, hw0, nhw, dst = units[i]
                    if dst[0] == "y7" and not done6:
                        combine6()
                        done6 = True
                    if kind != "V":
                        continue
                    s, w = dwait[i]
                    vector.wait_ge(dsems[s], w)
                    vector.reduce_sum(
                        dst_ap(units[i]), bufs[s][:, :, 0:nhw],
                        axis=mybir.AxisListType.X,
                    ).then_inc(red_sem, 1)
                # yt[:, 3, 7] = sum(ytx7); waits both engines' partial
                # writebacks (self red + cross act).
                vector.wait_ge(red_sem, RCOMB7 - 1)
                vector.wait_ge(act_sem, A_ROW7)
                vector.reduce_sum(
                    yt[:, CT - 1, BL - 1:BL], ytx7[:, :],
                    axis=mybir.AxisListType.X,
                ).then_inc(red_sem, 1)
                # Epilogue. y1ts copy (f32->bf16) runs on DVE.
                vector.wait_ge(pe_sem, 8)
                vector.tensor_copy(y1ts[:, :], y1t_ps[:, :]).then_inc(
                    red_sem, 1)
                vector.wait_ge(act_sem, A_EXP)
                vector.reciprocal(rs[:, :], ss[:, :]).then_inc(red_sem, 1)
                # t1 = (es * 1/s) * y1  (y1 read from PSUM); self-wait for
                # rs writeback.
                vector.wait_ge(red_sem, R_RECIP)
                vector.scalar_tensor_tensor(
                    t1s[:, :], es[:, :], rs[:, 0:1], y1_ps[:, :],
                    op0=mybir.AluOpType.mult, op1=mybir.AluOpType.mult,
                ).then_inc(red_sem, 1)
                vector.wait_ge(pe_sem, 9)
                vector.wait_ge(red_sem, R_STT)  # t1s writeback
                vector.tensor_add(y2s[:, :], t1s[:, :], p2_ps[:, :]).then_inc(
                    red_sem, 1)
                # Sigmoid tail, first half: outs_h1 = 0.5*tanh_h1 + 0.5
                vector.wait_ge(act_sem, A_TANH1)
                vector.tensor_scalar(
                    outs[:, 0:OP // 2], esig[:, 0:OP // 2], 0.5, 0.5,
                    op0=mybir.AluOpType.mult, op1=mybir.AluOpType.add,
                ).then_inc(red_sem, 1)

            @block.tensor
            def _(tensor):
                tensor.wait_ge(param_sem, NPARAM)
                # W1 matmuls per channel tile, issued as soon as that
                # tile of yt is fully reduced (overlaps the stream).
                for ct in range(CT):
                    if ct < CT - 1:
                        tensor.wait_ge(red_sem, ct_red[ct])
                        tensor.wait_ge(act_sem, ct_act[ct])
                    else:
                        tensor.wait_ge(red_sem, RCOMB7)
                        tensor.wait_ge(act_sem, NAS)
                    tensor.matmul(
                        y1_ps[:, :],
                        yt[:, ct, :],
                        w1ts[:, ct, :],
                        start=(ct == 0),
                        stop=(ct == CT - 1),
                    ).then_inc(pe_sem, 1)
                    tensor.matmul(
                        y1t_ps[:, :],
                        w1ts[:, ct, :],
                        yt[:, ct, :],
                        start=(ct == 0),
                        stop=(ct == CT - 1),
                    ).then_inc(pe_sem, 1)
                # p2[b, k] = sum_h y1T[h, b] * A2[h, k]
                tensor.wait_ge(red_sem, R_Y1TS)
                tensor.matmul(
                    p2_ps[:, :], y1ts[:, :], a2s[:, :], start=True, stop=True
                ).then_inc(pe_sem, 1)
                # w3*y2T via matmul with the w3-scaled identity
                tensor.wait_ge(red_sem, R_Y2)
                tensor.matmul(
                    y3t_ps[:, :], y2s[:, :], eyes[:, :], start=True, stop=True
                ).then_inc(pe_sem, 1)
                # out[b, o] = sum_h y3T[h, b] * W4T[h, o], in column
                # halves so the sigmoid tail pipelines across ACT/DVE.
                tensor.wait_ge(act_sem, A_RELU)
                tensor.matmul(
                    o_ps[:, 0:OP // 2], y3ts[:, :], w4ts[:, 0:OP // 2],
                    start=True, stop=True, skip_group_check=True,
                ).then_inc(pe_sem, 1)
                tensor.matmul(
                    o_ps[:, OP // 2:OP], y3ts[:, :], w4ts[:, OP // 2:OP],
                    start=True, stop=True, skip_group_check=True,
                ).then_inc(pe_sem, 1)

    return nc


def prep_in_maps(x, W1, A2, w2, w3, W4):
    """Shard x over batch; replicate (pre-transposed) params."""
    x = np.ascontiguousarray(np.asarray(x, dtype=np.float32))
    # W1T with the mean scale folded in: [c, h] -> [128, CT, HIDE] with
    # w1t[p, ct, h] = W1[h, ct*128+p] / hw
    w1t = np.ascontiguousarray(
        (np.asarray(W1, np.float32).T / HW).reshape(CT, 128, HIDE)
        .transpose(1, 0, 2)
    )
    import ml_dtypes

    a2 = np.ascontiguousarray(np.asarray(A2, np.float32)).astype(
        ml_dtypes.bfloat16)
    w4t = np.ascontiguousarray(np.asarray(W4, np.float32).T).astype(
        ml_dtypes.bfloat16)
    scal = np.empty((BL, 2), np.float32)
    scal[:, 0] = np.float32(w2)
    scal[:, 1] = np.float32(w3)
    # w3 folded into the transpose identity: the PE transpose-matmul then
    # produces w3*y2^T and the ACT copy applies relu.
    eye8 = (np.eye(BL) * np.float32(w3)).astype(ml_dtypes.bfloat16)

    in_maps = []
    for c in range(NCORES):
        xs = x[c * BL:(c + 1) * BL].reshape(BL, CT, 128, HW)
        in_maps.append(
            {
                "x": xs,
                "w1t": w1t,
                "a2": a2,
                "w4t": w4t,
                "scal": scal,
                "eye8": eye8,
            }
        )
    return in_maps


def run(inputs: dict, trace: bool = False, tmpdir: str | None = None,
        trace_cores=None):
    """Build + run on 8 cores. Returns (full_output, BassKernelResults)."""
    nc = build_nc()
    in_maps = prep_in_maps(
        inputs["x"], inputs["W1"], inputs["A2"], inputs["w2"], inputs["w3"],
        inputs["W4"],
    )
    res = run_bass_kernel_spmd(
        nc, in_maps, core_ids=list(range(NCORES)), trace=trace, tmpdir=tmpdir,
        trace_cores=trace_cores,
    )
    out = np.concatenate(
        [res.results[c]["out"] for c in range(NCORES)], axis=0)
    return out.reshape(B, OP, 1, 1).astype(np.float32), res


def kernel(**inputs) -> np.ndarray:
    out, _ = run(inputs, trace=False)
    return out
